# revision 7
# baseline (speedup 1.0000x reference)
"""CAM (channel-attention) module kernel for Trainium2.

Computes, per batch b:
    q      = x[b].reshape(C, H*W)
    E      = q @ q.T                                  # [C, C] channel Gram matrix
    A[i,j] = softmax_j(rowmax_i(E) - E[i,j])          # suppression softmax
           = exp(rowmin_i(E) - E[i,j]) / Z_i
    out[b] = gamma * (A @ q) + x[b]

Distribution: pure data-parallel over batch B=16 across 8 NeuronCores
(2 batches per core); gamma replicated. No collectives.

Per-core kernel strategy (v2):
  1. q loaded natural-layout [128, 4, 4096] exact fp32 (residual path needs
     the original bits). Batch-0's first load chunk is only 128 columns so
     the PE can start transposing ~3us earlier.
  2. qT built on-chip via PE transpose-mode in 128-column chunks. Each
     chunk's PSUM result is copied out twice: a float32r copy (ScalarE,
     rounding producer -> legal f32r matmul operand) used as the MOVING
     Gram operand, and a float16 copy (VectorE) used as the STATIONARY
     operand. fp16 stationaries enable fast weight loads, relieving the
     LDW port that was the gram-phase bottleneck; the f32r moving side
     keeps E at ~11-bit precision (fp16-both would double the E error).
  3. E computed block-upper-triangular (symmetry); strictly-lower 128x128
     blocks mirrored with exact fp32 PE transposes.
  4. S = exp(rowmin - E) fused on ScalarE (bias=rowmin, scale=-1) writing
     float16 directly, with accum_out producing Z in fp32.
  5. S transposed 128x128-blockwise on PE in fp16 -> ST (attention^T,
     stationary operand of the second matmul).
  6. U = ST.T @ qh on PE fully in fp16 (qh = fp16 cast of a q column
     chunk, cast 2 chunks ahead); epilogue out = (gamma/Z)*U + x is a
     single VectorE scalar_tensor_tensor reading the exact fp32 q and
     writing a float16 out tile. One aggregated 1MB store per s-group.
  7. Output DRAM tensor is float16 (host upcasts); halves store HBM
     traffic and store drain time. gamma=0 path stays exact to ~2^-11.
  8. Cross-batch software pipelining as before: batch b's transpose+Gram
     chunks are burst-interleaved with batch b-1's attention-apply so the
     PE never idles long enough for the HAM clock gate to re-throttle.
"""

import sys

import numpy as np

if "/opt/trn_rl_repo" not in sys.path:
    sys.path.insert(0, "/opt/trn_rl_repo")

B, C, H, W = 16, 512, 64, 64
N = H * W                # 4096 spatial positions
P = 128                  # partitions
CT = C // P              # 4 channel tiles
KT = N // P              # 32 contraction chunks for the Gram matmul
FD = 512                 # matmul moving free dim / PSUM bank width (fp32)
NCH = N // FD            # 8 output column chunks
N_CORES = 8
BPC = B // N_CORES       # 2 batches per core

# Moving-operand start column for the upper-triangular Gram matmul. Row-tile 3
# widens from 128 to 256 columns: float32r only streams at 1 cyc/row when the
# output free dim is >= 256, so recomputing block (3,2) is cheaper than a
# 128-wide f32r matmul.
MVSTART = [0, 128, 256, 256]

_CACHE = {}


def _build_nc():
    from contextlib import ExitStack

    import concourse.bacc as bacc
    import concourse.tile as tile
    from concourse import mybir
    from concourse.masks import make_identity

    f32 = mybir.dt.float32
    f32r = mybir.dt.float32r
    f16 = mybir.dt.float16
    AF = mybir.ActivationFunctionType
    ALU = mybir.AluOpType

    nc = bacc.Bacc(None, target_bir_lowering=False)
    # x stays float32 end-to-end on the load path: the DMA cast unit ROUNDS
    # when the destination dtype is float32r, which would corrupt the
    # residual. Reduced-precision matmul operands are produced by engine
    # cast-copies (ScalarE/VectorE).
    x_d = nc.dram_tensor("x", [BPC, C, N], f32, kind="ExternalInput")
    g_d = nc.dram_tensor("gamma", [1], f32, kind="ExternalInput")
    o_d = nc.dram_tensor("out", [BPC, C, N], f16, kind="ExternalOutput")

    with ExitStack() as ctx:
        tc = ctx.enter_context(tile.TileContext(nc))
        singles = ctx.enter_context(tc.tile_pool(name="singles", bufs=1))
        bigq = ctx.enter_context(tc.tile_pool(name="bigq", bufs=2))
        qtp = ctx.enter_context(tc.tile_pool(name="qtp", bufs=5))
        qrp = ctx.enter_context(tc.tile_pool(name="qrp", bufs=3))
        mats = ctx.enter_context(tc.tile_pool(name="mats", bufs=4))
        outp = ctx.enter_context(tc.tile_pool(name="outp", bufs=3))
        smallp = ctx.enter_context(tc.tile_pool(name="small", bufs=8))
        psp = ctx.enter_context(tc.tile_pool(name="ps", bufs=8, space="PSUM"))

        def ps_tile():
            return psp.tile([P, FD], f32, tag="ps", name="ps")

        LOOK = 2

        def emit_load(b, split_first=False):
            xb = x_d[b].rearrange("(ct p) n -> p ct n", p=P)
            ob = o_d[b].rearrange("(ct p) n -> p ct n", p=P)
            q = bigq.tile([P, CT, N], f32, tag="q")
            if split_first:
                # small first chunk so the first transposes start earlier
                nc.sync.dma_start(out=q[:, :, 0:P], in_=xb[:, :, 0:P])
                nc.sync.dma_start(out=q[:, :, P:FD], in_=xb[:, :, P:FD])
            else:
                nc.sync.dma_start(out=q[:, :, 0:FD], in_=xb[:, :, 0:FD])
            for s in range(1, NCH):
                nc.sync.dma_start(
                    out=q[:, :, s * FD : (s + 1) * FD],
                    in_=xb[:, :, s * FD : (s + 1) * FD],
                )
            return {"q": q, "xb": xb, "ob": ob}

        def emit_tr(st, k):
            q = st["q"]
            pst = psp.tile([P, FD], f32, tag="ps", name="pstr")
            for t in range(CT):
                nc.tensor.transpose(
                    pst[:, t * P : (t + 1) * P],
                    q[:, t, k * P : (k + 1) * P],
                    ident[:],
                )
            # rounding cast f32 -> f32r makes qk a legal f32r operand
            qk = qtp.tile([P, C], f32r, tag="qt")
            if k % 2 == 0:
                nc.scalar.copy(qk[:], pst[:])
            else:
                nc.vector.tensor_copy(qk[:], pst[:])
            st["qt"][k] = qk

        def emit_mm1(st, k):
            qkr = st["qt"][k]
            psE = st["psE"]
            for t in range(CT):
                w = C - MVSTART[t]
                nc.tensor.matmul(
                    psE[t][:, :w],
                    qkr[:, t * P : (t + 1) * P],
                    qkr[:, MVSTART[t] :],
                    start=(k == 0),
                    stop=(k == KT - 1),
                )

        def emit_cast(st, s):
            q = st["q"]
            qr = qrp.tile([P, CT, FD], f16, tag="qr")
            if s % 4 == 3:
                nc.vector.tensor_copy(qr[:], q[:, :, s * FD : (s + 1) * FD])
            else:
                nc.scalar.copy(qr[:], q[:, :, s * FD : (s + 1) * FD])
            st["qrs"][s] = qr

        def emit_mm2_s(st, s):
            # one s-chunk of mm2 + epilogue: 4 psU groups, 1 aggregated store
            if s == 0:
                emit_cast(st, 0)
                emit_cast(st, 1)
            if s + 2 < NCH:
                emit_cast(st, s + 2)
            qr = st["qrs"][s]
            q, ob, ST, grz = st["q"], st["ob"], st["ST"], st["grz"]
            ot = outp.tile([P, CT, FD], f16, tag="ot")
            for t in range(CT):
                pu = ps_tile()
                for jt in range(CT):
                    nc.tensor.matmul(
                        pu[:],
                        ST[jt][:, t * P : (t + 1) * P],
                        qr[:, jt, :],
                        start=(jt == 0),
                        stop=(jt == CT - 1),
                    )
                # out = (U * gamma/Z) + x in one VectorE op, fp16 out
                nc.vector.scalar_tensor_tensor(
                    ot[:, t, :],
                    pu[:],
                    grz[t][:],
                    q[:, t, s * FD : (s + 1) * FD],
                    op0=ALU.mult,
                    op1=ALU.add,
                )
            nc.sync.dma_start(out=ob[:, :, s * FD : (s + 1) * FD], in_=ot[:])

        def emit_gram(st, prev, skip_chunks=0):
            """Transposes + Gram matmul for `st`, burst-interleaved with the
            previous batch's attention-apply (mm2) so PE never idles long
            enough for the HAM clock gate to re-throttle."""
            st["psE"] = [ps_tile() for _ in range(CT)]
            if "qt" not in st:
                st["qt"] = [None] * KT
            # chunks below skip_chunks had their transposes emitted during the
            # previous batch's softmax phase; emit their (lookahead-deferred)
            # Gram matmuls now so the accumulation starts with chunk 0
            for kk in range(max(0, skip_chunks - LOOK)):
                emit_mm1(st, kk)
            # chunks processed in PAIRS (2x4 transposes, then 2x4 Gram
            # matmuls): transpose-mode <-> regular-mode switches flush the
            # PE pipeline, so longer same-mode runs pipeline better
            for k in range(skip_chunks, KT, 2):
                emit_tr(st, k)
                emit_tr(st, k + 1)
                if k >= LOOK:
                    emit_mm1(st, k - 2)
                    emit_mm1(st, k - 1)
                # only 6 of 8 s-groups here: the last two fill this batch's
                # own softmax phase, where the PE would otherwise idle
                if (
                    prev is not None
                    and k >= 6
                    and (k - 6) % 4 == 0
                    and (k - 6) // 4 < NCH - 2
                ):
                    emit_mm2_s(prev, (k - 6) // 4)
            for k in range(KT - 2, KT):
                emit_mm1(st, k)

        def emit_softmax(st, prev=None):
            # ---- copy E out of PSUM; mirror strictly-lower blocks ----
            psE = st["psE"]
            E = []
            for t in range(CT):
                e = mats.tile([P, FD], f32, tag="E")
                w = C - MVSTART[t]
                if t % 2 == 0:
                    nc.scalar.copy(e[:, MVSTART[t] :], psE[t][:, :w])
                else:
                    nc.vector.tensor_copy(e[:, MVSTART[t] :], psE[t][:, :w])
                E.append(e)
            # E[t][:, s-block] = E[s][:, t-block].T for s < t (exact fp32
            # transposes: E magnitudes are ~4e3 and feed exp directly, so
            # low-precision rounding here would be a real error).
            for t in range(1, CT):
                for s in range(t):
                    if t == 3 and s == 2:
                        continue  # computed directly via the widened row-tile 3
                    pm = ps_tile()
                    nc.tensor.transpose(
                        pm[:, :P], E[s][:, t * P : (t + 1) * P], ident[:]
                    )
                    if (t + s) % 2 == 0:
                        nc.scalar.copy(E[t][:, s * P : (s + 1) * P], pm[:, :P])
                    else:
                        nc.vector.tensor_copy(
                            E[t][:, s * P : (s + 1) * P], pm[:, :P]
                        )

            # deferred mm2 s-group of the previous batch keeps the PE busy
            # while the rowmin/exp chains run on VectorE/ScalarE; for the
            # first batch, the NEXT batch's first transposes fill in instead
            if prev is not None:
                emit_mm2_s(prev, NCH - 2)
            elif st.get("next") is not None:
                emit_tr(st["next"], 0)
                emit_tr(st["next"], 1)

            # ---- suppression softmax: S = exp(rowmin - E), Z = rowsum(S),
            # S written as fp16 (legal fast-weight-load transpose operand) ----
            S = []
            grz = []
            for t in range(CT):
                rm = smallp.tile([P, 1], f32, tag="rm")
                nc.vector.tensor_reduce(
                    rm[:], E[t][:], axis=mybir.AxisListType.X, op=ALU.min
                )
                s_t = mats.tile([P, FD], f16, tag="S")
                z = smallp.tile([P, 1], f32, tag="z")
                nc.scalar.activation(
                    s_t[:], E[t][:], AF.Exp, bias=rm[:], scale=-1.0, accum_out=z[:]
                )
                S.append(s_t)
                rz = smallp.tile([P, 1], f32, tag="rz")
                nc.vector.reciprocal(rz[:], z[:])
                g = smallp.tile([P, 1], f32, tag="grz")
                nc.vector.tensor_mul(g[:], rz[:], gam[:])
                grz.append(g)

            if prev is not None:
                emit_mm2_s(prev, NCH - 1)
            elif st.get("next") is not None:
                emit_tr(st["next"], 2)
                emit_tr(st["next"], 3)

            # ---- ST = S.T (attention^T), 128x128 fp16 blocks on PE ----
            # Ordered by source tile t so each ST transpose can start as soon
            # as S[t] exists; 4 PSUM banks stay open across the t loop.
            pstS = [
                psp.tile([P, FD], f16, tag="ps", name="pstS") for _ in range(CT)
            ]
            for t in range(CT):
                for jt in range(CT):
                    nc.tensor.transpose(
                        pstS[jt][:, t * P : (t + 1) * P],
                        S[t][:, jt * P : (jt + 1) * P],
                        identh[:],
                    )
            ST = []
            for jt in range(CT):
                stj = mats.tile([P, FD], f16, tag="ST")
                if jt % 2 == 0:
                    nc.scalar.copy(stj[:], pstS[jt][:])
                else:
                    nc.vector.tensor_copy(stj[:], pstS[jt][:])
                ST.append(stj)
            st["ST"] = ST
            st["grz"] = grz
            st["qrs"] = [None] * NCH

        # ---- pipelined driver: batch b's Gram phase overlaps batch b-1's
        # attention-apply phase on the PE ----
        st0 = emit_load(0, split_first=True)
        st1 = emit_load(1)

        ident = singles.tile([P, P], f32)
        make_identity(nc, ident)
        identh = singles.tile([P, P], f16)
        nc.vector.tensor_copy(identh[:], ident[:])

        # gamma broadcast to all partitions as a per-partition scalar
        gam = singles.tile([P, 1], f32)
        nc.gpsimd.dma_start(out=gam[:], in_=g_d[:].to_broadcast([P, 1]))

        emit_gram(st0, None)
        st1["qt"] = [None] * KT
        st0["next"] = st1
        emit_softmax(st0, None)
        emit_gram(st1, st0, skip_chunks=4)
        emit_softmax(st1, st0)
        for s in range(NCH):
            emit_mm2_s(st1, s)

    nc.compile()
    return nc


def _get_nc():
    if "nc" not in _CACHE:
        _CACHE["nc"] = _build_nc()
    return _CACHE["nc"]


def kernel(x: np.ndarray, gamma: np.ndarray) -> np.ndarray:
    from concourse.bass_utils import run_bass_kernel_spmd

    nc = _get_nc()
    x = np.ascontiguousarray(np.asarray(x, dtype=np.float32))
    gamma = np.ascontiguousarray(np.asarray(gamma, dtype=np.float32))
    xs = x.reshape(B, C, N)
    in_maps = [
        {
            "x": np.ascontiguousarray(xs[c * BPC : (c + 1) * BPC]),
            "gamma": gamma,
        }
        for c in range(N_CORES)
    ]
    res = run_bass_kernel_spmd(nc, in_maps, core_ids=list(range(N_CORES)))
    out = np.stack(
        [np.asarray(res.results[c]["out"]) for c in range(N_CORES)], axis=0
    )
    return out.reshape(B, C, H, W).astype(np.float32)


# revision 9
# speedup vs baseline: 1.0111x; 1.0111x over previous
"""CAM (channel-attention) module kernel for Trainium2.

Computes, per batch b:
    q      = x[b].reshape(C, H*W)
    E      = q @ q.T                                  # [C, C] channel Gram matrix
    A[i,j] = softmax_j(rowmax_i(E) - E[i,j])          # suppression softmax
           = exp(rowmin_i(E) - E[i,j]) / Z_i
    out[b] = gamma * (A @ q) + x[b]

Distribution: pure data-parallel over batch B=16 across 8 NeuronCores
(2 batches per core); gamma replicated. No collectives.

Per-core kernel strategy (v2):
  1. q loaded natural-layout [128, 4, 4096] exact fp32 (residual path needs
     the original bits). Batch-0's first load chunk is only 128 columns so
     the PE can start transposing ~3us earlier.
  2. qT built on-chip via PE transpose-mode in 128-column chunks. Each
     chunk's PSUM result is copied out twice: a float32r copy (ScalarE,
     rounding producer -> legal f32r matmul operand) used as the MOVING
     Gram operand, and a float16 copy (VectorE) used as the STATIONARY
     operand. fp16 stationaries enable fast weight loads, relieving the
     LDW port that was the gram-phase bottleneck; the f32r moving side
     keeps E at ~11-bit precision (fp16-both would double the E error).
  3. E computed block-upper-triangular (symmetry); strictly-lower 128x128
     blocks mirrored with exact fp32 PE transposes.
  4. S = exp(rowmin - E) fused on ScalarE (bias=rowmin, scale=-1) writing
     float16 directly, with accum_out producing Z in fp32.
  5. S transposed 128x128-blockwise on PE in fp16 -> ST (attention^T,
     stationary operand of the second matmul).
  6. U = ST.T @ qh on PE fully in fp16 (qh = fp16 cast of a q column
     chunk, cast 2 chunks ahead); epilogue out = (gamma/Z)*U + x is a
     single VectorE scalar_tensor_tensor reading the exact fp32 q and
     writing a float16 out tile. One aggregated 1MB store per s-group.
  7. Output DRAM tensor is float16 (host upcasts); halves store HBM
     traffic and store drain time. gamma=0 path stays exact to ~2^-11.
  8. Cross-batch software pipelining as before: batch b's transpose+Gram
     chunks are burst-interleaved with batch b-1's attention-apply so the
     PE never idles long enough for the HAM clock gate to re-throttle.
"""

import sys

import numpy as np

if "/opt/trn_rl_repo" not in sys.path:
    sys.path.insert(0, "/opt/trn_rl_repo")

B, C, H, W = 16, 512, 64, 64
N = H * W                # 4096 spatial positions
P = 128                  # partitions
CT = C // P              # 4 channel tiles
KT = N // P              # 32 contraction chunks for the Gram matmul
FD = 512                 # matmul moving free dim / PSUM bank width (fp32)
NCH = N // FD            # 8 output column chunks
N_CORES = 8
BPC = B // N_CORES       # 2 batches per core

# Moving-operand start column for the upper-triangular Gram matmul. Row-tile 3
# widens from 128 to 256 columns: float32r only streams at 1 cyc/row when the
# output free dim is >= 256, so recomputing block (3,2) is cheaper than a
# 128-wide f32r matmul.
MVSTART = [0, 128, 256, 256]

_CACHE = {}


def _build_nc():
    from contextlib import ExitStack

    import concourse.bacc as bacc
    import concourse.tile as tile
    from concourse import mybir
    from concourse.masks import make_identity

    f32 = mybir.dt.float32
    f32r = mybir.dt.float32r
    f16 = mybir.dt.float16
    AF = mybir.ActivationFunctionType
    ALU = mybir.AluOpType

    nc = bacc.Bacc(None, target_bir_lowering=False)
    # x stays float32 end-to-end on the load path: the DMA cast unit ROUNDS
    # when the destination dtype is float32r, which would corrupt the
    # residual. Reduced-precision matmul operands are produced by engine
    # cast-copies (ScalarE/VectorE).
    x_d = nc.dram_tensor("x", [BPC, C, N], f32, kind="ExternalInput")
    g_d = nc.dram_tensor("gamma", [1], f32, kind="ExternalInput")
    o_d = nc.dram_tensor("out", [BPC, C, N], f16, kind="ExternalOutput")

    with ExitStack() as ctx:
        tc = ctx.enter_context(tile.TileContext(nc))
        singles = ctx.enter_context(tc.tile_pool(name="singles", bufs=1))
        bigq = ctx.enter_context(tc.tile_pool(name="bigq", bufs=2))
        qtp = ctx.enter_context(tc.tile_pool(name="qtp", bufs=5))
        qrp = ctx.enter_context(tc.tile_pool(name="qrp", bufs=3))
        mats = ctx.enter_context(tc.tile_pool(name="mats", bufs=4))
        outp = ctx.enter_context(tc.tile_pool(name="outp", bufs=3))
        smallp = ctx.enter_context(tc.tile_pool(name="small", bufs=8))
        psp = ctx.enter_context(tc.tile_pool(name="ps", bufs=8, space="PSUM"))

        def ps_tile():
            return psp.tile([P, FD], f32, tag="ps", name="ps")

        LOOK = 2

        def emit_load(b, split_first=False):
            xb = x_d[b].rearrange("(ct p) n -> p ct n", p=P)
            ob = o_d[b].rearrange("(ct p) n -> p ct n", p=P)
            q = bigq.tile([P, CT, N], f32, tag="q")
            if split_first:
                # two half chunks so the first transposes start earlier and
                # chunk pair (2,3) doesn't stall on one big tail transfer
                nc.sync.dma_start(out=q[:, :, 0 : 2 * P], in_=xb[:, :, 0 : 2 * P])
                nc.sync.dma_start(out=q[:, :, 2 * P : FD], in_=xb[:, :, 2 * P : FD])
            else:
                nc.sync.dma_start(out=q[:, :, 0:FD], in_=xb[:, :, 0:FD])
            for s in range(1, NCH):
                nc.sync.dma_start(
                    out=q[:, :, s * FD : (s + 1) * FD],
                    in_=xb[:, :, s * FD : (s + 1) * FD],
                )
            return {"q": q, "xb": xb, "ob": ob}

        def emit_tr(st, k):
            q = st["q"]
            pst = psp.tile([P, FD], f32, tag="ps", name="pstr")
            for t in range(CT):
                nc.tensor.transpose(
                    pst[:, t * P : (t + 1) * P],
                    q[:, t, k * P : (k + 1) * P],
                    ident[:],
                )
            # rounding cast f32 -> f32r makes qk a legal f32r operand
            qk = qtp.tile([P, C], f32r, tag="qt")
            if k % 2 == 0:
                nc.scalar.copy(qk[:], pst[:])
            else:
                nc.vector.tensor_copy(qk[:], pst[:])
            st["qt"][k] = qk

        def emit_mm1(st, k):
            qkr = st["qt"][k]
            psE = st["psE"]
            for t in range(CT):
                w = C - MVSTART[t]
                nc.tensor.matmul(
                    psE[t][:, :w],
                    qkr[:, t * P : (t + 1) * P],
                    qkr[:, MVSTART[t] :],
                    start=(k == 0),
                    stop=(k == KT - 1),
                )

        def emit_cast(st, s):
            q = st["q"]
            qr = qrp.tile([P, CT, FD], f16, tag="qr")
            nc.vector.tensor_copy(qr[:], q[:, :, s * FD : (s + 1) * FD])
            st["qrs"][s] = qr

        def emit_mm2_s(st, s):
            # one s-chunk of mm2 + epilogue: 4 psU groups, 1 aggregated store
            if s == 0:
                emit_cast(st, 0)
                emit_cast(st, 1)
            if s + 2 < NCH:
                emit_cast(st, s + 2)
            qr = st["qrs"][s]
            q, ob, ST, grz = st["q"], st["ob"], st["ST"], st["grz"]
            ot = outp.tile([P, CT, FD], f16, tag="ot")
            for t in range(CT):
                pu = ps_tile()
                for jt in range(CT):
                    nc.tensor.matmul(
                        pu[:],
                        ST[jt][:, t * P : (t + 1) * P],
                        qr[:, jt, :],
                        start=(jt == 0),
                        stop=(jt == CT - 1),
                    )
                if t < 2:
                    # split epilogue across ScalarE (PSUM in-place scale) and
                    # VectorE (residual add) so VectorE doesn't pace the PE
                    nc.scalar.mul(pu[:], pu[:], grz[t][:])
                    nc.vector.tensor_add(
                        ot[:, t, :], pu[:], q[:, t, s * FD : (s + 1) * FD]
                    )
                else:
                    # out = (U * gamma/Z) + x in one VectorE op, fp16 out
                    nc.vector.scalar_tensor_tensor(
                        ot[:, t, :],
                        pu[:],
                        grz[t][:],
                        q[:, t, s * FD : (s + 1) * FD],
                        op0=ALU.mult,
                        op1=ALU.add,
                    )
            nc.sync.dma_start(out=ob[:, :, s * FD : (s + 1) * FD], in_=ot[:])

        def emit_gram(st, prev, skip_chunks=0):
            """Transposes + Gram matmul for `st`, burst-interleaved with the
            previous batch's attention-apply (mm2) so PE never idles long
            enough for the HAM clock gate to re-throttle."""
            st["psE"] = [ps_tile() for _ in range(CT)]
            if "qt" not in st:
                st["qt"] = [None] * KT
            # chunks below skip_chunks had their transposes emitted during the
            # previous batch's softmax phase; emit their (lookahead-deferred)
            # Gram matmuls now so the accumulation starts with chunk 0
            for kk in range(max(0, skip_chunks - LOOK)):
                emit_mm1(st, kk)
            # chunks processed in PAIRS (2x4 transposes, then 2x4 Gram
            # matmuls): transpose-mode <-> regular-mode switches flush the
            # PE pipeline, so longer same-mode runs pipeline better
            for k in range(skip_chunks, KT, 2):
                emit_tr(st, k)
                emit_tr(st, k + 1)
                if k >= LOOK:
                    emit_mm1(st, k - 2)
                    emit_mm1(st, k - 1)
                # only 6 of 8 s-groups here: the last two fill this batch's
                # own softmax phase, where the PE would otherwise idle
                if (
                    prev is not None
                    and k >= 6
                    and (k - 6) % 4 == 0
                    and (k - 6) // 4 < NCH - 3
                ):
                    emit_mm2_s(prev, (k - 6) // 4)
            for k in range(KT - 2, KT):
                emit_mm1(st, k)

        def emit_softmax(st, prev=None):
            # ---- copy E out of PSUM; mirror strictly-lower blocks ----
            psE = st["psE"]
            E = []
            for t in range(CT):
                e = mats.tile([P, FD], f32, tag="E")
                w = C - MVSTART[t]
                if t % 2 == 0:
                    nc.scalar.copy(e[:, MVSTART[t] :], psE[t][:, :w])
                else:
                    nc.vector.tensor_copy(e[:, MVSTART[t] :], psE[t][:, :w])
                E.append(e)
            # E[t][:, s-block] = E[s][:, t-block].T for s < t (exact fp32
            # transposes: E magnitudes are ~4e3 and feed exp directly, so
            # low-precision rounding here would be a real error).
            for t in range(1, CT):
                for s in range(t):
                    if t == 3 and s == 2:
                        continue  # computed directly via the widened row-tile 3
                    pm = ps_tile()
                    nc.tensor.transpose(
                        pm[:, :P], E[s][:, t * P : (t + 1) * P], ident[:]
                    )
                    if (t + s) % 2 == 0:
                        nc.scalar.copy(E[t][:, s * P : (s + 1) * P], pm[:, :P])
                    else:
                        nc.vector.tensor_copy(
                            E[t][:, s * P : (s + 1) * P], pm[:, :P]
                        )

            # deferred mm2 s-group of the previous batch keeps the PE busy
            # while the rowmin/exp chains run on VectorE/ScalarE; for the
            # first batch, the NEXT batch's first transposes fill in instead
            if prev is not None:
                emit_mm2_s(prev, NCH - 3)
            elif st.get("next") is not None:
                emit_tr(st["next"], 0)
                emit_tr(st["next"], 1)

            # ---- suppression softmax: S = exp(rowmin - E), Z = rowsum(S),
            # S written as fp16 (legal fast-weight-load transpose operand) ----
            S = []
            grz = []
            for t in range(CT):
                rm = smallp.tile([P, 1], f32, tag="rm")
                nc.vector.tensor_reduce(
                    rm[:], E[t][:], axis=mybir.AxisListType.X, op=ALU.min
                )
                s_t = mats.tile([P, FD], f16, tag="S")
                z = smallp.tile([P, 1], f32, tag="z")
                nc.scalar.activation(
                    s_t[:], E[t][:], AF.Exp, bias=rm[:], scale=-1.0, accum_out=z[:]
                )
                S.append(s_t)
                rz = smallp.tile([P, 1], f32, tag="rz")
                nc.vector.reciprocal(rz[:], z[:])
                g = smallp.tile([P, 1], f32, tag="grz")
                nc.vector.tensor_mul(g[:], rz[:], gam[:])
                grz.append(g)

            if prev is not None:
                emit_mm2_s(prev, NCH - 2)
            elif st.get("next") is not None:
                emit_tr(st["next"], 2)
                emit_tr(st["next"], 3)

            # ---- ST = S.T (attention^T), 128x128 fp16 blocks on PE ----
            # Ordered by source tile t so each ST transpose can start as soon
            # as S[t] exists; 4 PSUM banks stay open across the t loop.
            pstS = [
                psp.tile([P, FD], f16, tag="ps", name="pstS") for _ in range(CT)
            ]
            for t in range(CT):
                for jt in range(CT):
                    nc.tensor.transpose(
                        pstS[jt][:, t * P : (t + 1) * P],
                        S[t][:, jt * P : (jt + 1) * P],
                        identh[:],
                    )
            if prev is not None:
                emit_mm2_s(prev, NCH - 1)
            ST = []
            for jt in range(CT):
                stj = mats.tile([P, FD], f16, tag="ST")
                if jt % 2 == 0:
                    nc.scalar.copy(stj[:], pstS[jt][:])
                else:
                    nc.vector.tensor_copy(stj[:], pstS[jt][:])
                ST.append(stj)
            st["ST"] = ST
            st["grz"] = grz
            st["qrs"] = [None] * NCH

        # ---- pipelined driver: batch b's Gram phase overlaps batch b-1's
        # attention-apply phase on the PE ----
        st0 = emit_load(0, split_first=True)
        st1 = emit_load(1)

        ident = singles.tile([P, P], f32)
        make_identity(nc, ident)
        identh = singles.tile([P, P], f16)
        nc.vector.tensor_copy(identh[:], ident[:])

        # dummy transposes ramp the PE while the first load chunk is still in
        # flight, so the HAM clock gate reaches 8/8 before real work starts
        # (a cold PE runs the first ~3.4us of the Gram phase at half clock)
        for _ in range(7):
            pw = psp.tile([P, P], f32, tag="ps", name="warm")
            for _r in range(4):
                nc.tensor.transpose(pw[:], ident[:], ident[:])

        # gamma broadcast to all partitions as a per-partition scalar
        gam = singles.tile([P, 1], f32)
        nc.gpsimd.dma_start(out=gam[:], in_=g_d[:].to_broadcast([P, 1]))

        emit_gram(st0, None)
        st1["qt"] = [None] * KT
        st0["next"] = st1
        emit_softmax(st0, None)
        emit_gram(st1, st0, skip_chunks=4)
        emit_softmax(st1, st0)
        for s in range(NCH):
            emit_mm2_s(st1, s)

    nc.compile()
    return nc


def _get_nc():
    if "nc" not in _CACHE:
        _CACHE["nc"] = _build_nc()
    return _CACHE["nc"]


def kernel(x: np.ndarray, gamma: np.ndarray) -> np.ndarray:
    from concourse.bass_utils import run_bass_kernel_spmd

    nc = _get_nc()
    x = np.ascontiguousarray(np.asarray(x, dtype=np.float32))
    gamma = np.ascontiguousarray(np.asarray(gamma, dtype=np.float32))
    xs = x.reshape(B, C, N)
    in_maps = [
        {
            "x": np.ascontiguousarray(xs[c * BPC : (c + 1) * BPC]),
            "gamma": gamma,
        }
        for c in range(N_CORES)
    ]
    res = run_bass_kernel_spmd(nc, in_maps, core_ids=list(range(N_CORES)))
    out = np.stack(
        [np.asarray(res.results[c]["out"]) for c in range(N_CORES)], axis=0
    )
    return out.reshape(B, C, H, W).astype(np.float32)
